# revision 5
# baseline (speedup 1.0000x reference)
"""Trainium2 Bass kernel for nn_DAGModel_88630945120510 (gnn_message_passing).

Data-parallel over batch: 32 batches -> 8 cores x 4 batches. Node buffer in
DRAM as [TOTAL, BL*H] bf16 (1KB rows). Per depth: multi-queue dma_gather of
compacted parent rows (token-major) -> parent-sum via identity-matmul PSUM
accumulation (fp32) -> PE transposes to feature-major -> bf16 MLP with ACT
bias/relu -> transpose back -> DVE residual -> contiguous DMA write.

Depth nodes are split into 2 age-sorted chunks (by newest referenced parent)
so most of the next depth's gathers issue while the current depth computes.
Host side does input marshalling only: index compaction/permutation, bf16
casts, and output un-permutation.
"""

import hashlib
import numpy as np
import ml_dtypes
from contextlib import ExitStack

import concourse.bass as bass
import concourse.mybir as mybir
import concourse.tile as tile
from concourse import bacc
from concourse.bass_utils import run_bass_kernel_spmd
from concourse._compat import cdiv

F32 = mybir.dt.float32
BF16 = mybir.dt.bfloat16
I16 = mybir.dt.int16

B, H, E = 32, 128, 64
D, N, P = 32, 512, 8
TOTAL = 2 + D * N
NCORES = 8
BL = B // NCORES          # batches per core
ROW = BL * H              # elems per DRAM row record (bf16)
NCHUNK = 2                # age-sorted chunks per depth
CN = N // NCHUNK          # nodes per chunk
MAXCALL = 768             # idx per gather call (descriptor-ring limit < 1024)
NQ = 4                    # SWDGE queues

bf16 = ml_dtypes.bfloat16


# ----------------------------------------------------------------------------
# host-side layout builder
# ----------------------------------------------------------------------------

def build_layout(parent_idx):
    parent_idx = np.asarray(parent_idx)
    assert parent_idx.shape == (D, N, P)
    pos = np.zeros((D, N), np.int64)   # pos[d][n] = storage slot of node n
    sig = np.zeros((D, N), np.int64)   # sig[d][j] = node stored at slot j
    rowmap = np.zeros(TOTAL, np.int64)
    rowmap[0], rowmap[1] = 0, 1
    steps = []
    packed_cols = []
    col_off = 0
    for d in range(D):
        bound = 2 + d * N
        pv = parent_idx[d].astype(np.int64).copy()   # [N, P]
        pv[(pv < 0) | (pv >= bound)] = 0             # pad/OOB -> zero row
        rows = rowmap[pv]                            # [N, P] storage rows
        maxrow = rows.max(axis=1)
        k = (pv != 0).sum(1)
        # age sort: oldest-referencing nodes first -> chunk 0 prefetchable
        order = np.lexsort((-k, maxrow))
        chunks = []
        for c in range(NCHUNK):
            csel = order[c * CN:(c + 1) * CN]
            # within chunk sort by #parents desc for segment-prefix property
            csel = csel[np.argsort(-k[csel], kind="stable")]
            chunks.append(csel)
        order = np.concatenate(chunks)
        sig[d] = order
        pos[d, order] = np.arange(N)
        rowmap[bound: bound + N] = bound + pos[d]

        step_chunks = []
        for c in range(NCHUNK):
            csel = chunks[c]
            crows = rows[csel]                       # [CN, P]
            ck = k[csel]
            # compact nonzero parents to the left
            comp = np.zeros_like(crows)
            for j in range(CN):
                nz = crows[j][crows[j] != 0]
                comp[j, : len(nz)] = nz
            # p-major stream, each segment padded to 128
            segs = []
            cur_len = 0
            pieces = []   # (bank_j, stream_block_index)
            for p in range(P):
                cp = int((ck > p).sum())
                if cp == 0:
                    continue
                cpad = cdiv(cp, 128) * 128
                seg = np.zeros(cpad, np.int64)
                seg[:cp] = comp[:cp, p]
                blk0 = cur_len // 128
                segs.append(seg)
                cur_len += cpad
                for jj in range(cpad // 128):
                    pieces.append((jj, blk0 + jj))
            stream = np.concatenate(segs)
            L = len(stream)
            assert L % 128 == 0
            nblk = L // 128
            # split into calls of <= MAXCALL at block boundaries
            calls = []
            b0 = 0
            while b0 < nblk:
                nb = min(MAXCALL // 128, nblk - b0)
                calls.append((b0, nb, col_off))
                seg_idx = stream[b0 * 128:(b0 + nb) * 128]
                pk = seg_idx.astype(np.int16).reshape(-1, 16).T  # [16, nb*8]
                packed_cols.append(np.tile(pk, (8, 1)))
                col_off += nb * 8
                b0 += nb
            # piece -> (bank, call, block-in-call, start, stop)
            ops = []
            first_for_bank = {}
            for (jj, blk) in pieces:
                bank = c * NCHUNK + jj if NCHUNK == 2 else c * 2 + jj
                ci = 0
                while not (calls[ci][0] <= blk < calls[ci][0] + calls[ci][1]):
                    ci += 1
                ops.append([bank, ci, blk - calls[ci][0], False, False])
                if bank not in first_for_bank:
                    first_for_bank[bank] = ops[-1]
                ops[-1][3] = False
            for bank, op in first_for_bank.items():
                op[3] = True
            last_for_bank = {}
            for op in ops:
                last_for_bank[op[0]] = op
            for bank, op in last_for_bank.items():
                op[4] = True
            # prefix of rows this chunk reads (for precise RAW deps)
            cbound = int(maxrow[csel].max()) + 1
            step_chunks.append(dict(calls=calls, ops=ops, nblk=nblk,
                                    cbound=cbound))
        steps.append(dict(chunks=step_chunks, bound=bound))

    idx_packed = np.concatenate(packed_cols, axis=1)   # [128, col_off]
    return dict(steps=steps, idx_packed=idx_packed, pos=pos, sig=sig,
                total_cols=col_off)


# ----------------------------------------------------------------------------
# device kernel
# ----------------------------------------------------------------------------

def build_nc(meta, repeat=1):
    nc = bacc.Bacc("TRN2", target_bir_lowering=False, debug=False,
                   num_swdge_queues=NQ)

    bufrows = nc.declare_dram_parameter("bufrows", [TOTAL, BL, H], BF16,
                                        isOutput=True)
    init2 = nc.declare_dram_parameter("init2", [2, BL, H], BF16, isOutput=False)
    embt = nc.declare_dram_parameter("embt", [E, D * N], BF16, isOutput=False)
    w1pv_d = nc.declare_dram_parameter("w1pv", [H, H], BF16, isOutput=False)
    w1ne_d = nc.declare_dram_parameter("w1ne", [E, H], BF16, isOutput=False)
    w2t_d = nc.declare_dram_parameter("w2t", [H, H], BF16, isOutput=False)
    b1_d = nc.declare_dram_parameter("b1", [H, 1], F32, isOutput=False)
    b2_d = nc.declare_dram_parameter("b2", [H, 1], F32, isOutput=False)
    ident_d = nc.declare_dram_parameter("ident", [H, H], BF16, isOutput=False)
    idxs_d = nc.declare_dram_parameter("idxs", [128, meta["total_cols"]], I16,
                                       isOutput=False)

    steps = meta["steps"]
    qn = [0]  # global queue round-robin

    with tile.TileContext(nc) as tc, ExitStack() as ctx:
        const = ctx.enter_context(tc.tile_pool(name="const", bufs=1))
        gpool = ctx.enter_context(tc.tile_pool(name="g", bufs=10))
        spool = ctx.enter_context(tc.tile_pool(name="s", bufs=2))
        nepool = ctx.enter_context(tc.tile_pool(name="ne", bufs=3))
        psum = ctx.enter_context(tc.tile_pool(name="ps", bufs=1, space="PSUM"))

        idxs_sb = const.tile([128, meta["total_cols"]], I16, tag="idxs")
        nc.sync.dma_start(idxs_sb[:], idxs_d[:])
        w1pv = const.tile([H, H], BF16, tag="w1pv")
        nc.sync.dma_start(w1pv[:], w1pv_d[:])
        w1ne = const.tile([E, H], BF16, tag="w1ne")
        nc.sync.dma_start(w1ne[:], w1ne_d[:])
        w2t = const.tile([H, H], BF16, tag="w2t")
        nc.sync.dma_start(w2t[:], w2t_d[:])
        b1 = const.tile([H, 1], F32, tag="b1")
        nc.sync.dma_start(b1[:], b1_d[:])
        b2 = const.tile([H, 1], F32, tag="b2")
        nc.sync.dma_start(b2[:], b2_d[:])
        ident = const.tile([H, H], BF16, tag="ident")
        nc.sync.dma_start(ident[:], ident_d[:])

        nc.sync.dma_start(bufrows[0:2, :, :], init2[:])

        NB = MAXCALL // 128

        def emit_gather(d, c, gtiles):
            s = steps[d]["chunks"][c]
            src = bufrows[0:s["cbound"], :, :].rearrange("r b h -> r (b h)")
            tiles = []
            for (b0, nb, coff) in s["calls"]:
                g = gpool.tile([128, NB, ROW], BF16, tag="g")
                nidx = nb * 128
                nc.gpsimd.dma_gather(
                    g[:, 0:nb, :], src, idxs_sb[:, coff:coff + nidx // 16],
                    nidx, nidx, ROW, queue_num=qn[0] % NQ)
                qn[0] += 1
                tiles.append(g)
            gtiles[(d, c)] = tiles

        def emit_compute(d, gtiles):
            s = steps[d]
            bound = s["bound"]

            ne_t = nepool.tile([E, N], BF16, tag="ne")
            nc.sync.dma_start(ne_t[:], embt[:, d * N:(d + 1) * N])

            ps = psum.tile([128, 8, 512], F32, tag="ps")

            # parent-sum: identity matmuls accumulate bank k (fp32)
            for c in range(NCHUNK):
                ch = s["chunks"][c]
                tiles = gtiles.pop((d, c))
                for (bank, ci, blk, start, stop) in ch["ops"]:
                    nc.tensor.matmul(ps[:, bank, :], ident[:],
                                     tiles[ci][:, blk, :],
                                     start=start, stop=stop)

            # pv tm -> sbuf bf16 (residual source + transpose input)
            pv_sb = spool.tile([128, 4, 512], BF16, tag="pv")
            nc.scalar.activation(pv_sb[:], ps[:, 0:4, :],
                                 mybir.ActivationFunctionType.Copy)

            # transposes: pv fm per batch into banks 4..7 (bf16 views)
            psT = ps[:, 4:8, :].rearrange("p a e -> p (a e)").bitcast(
                BF16).rearrange("p (a e) -> p a e", a=4)   # [128, 4, 1024]
            for b in range(BL):
                for kk in range(4):
                    nc.tensor.transpose(
                        psT[:, b, kk * 128:(kk + 1) * 128],
                        pv_sb[:, kk, b * H:(b + 1) * H], ident[:])
            pvT = spool.tile([128, 4, 512], BF16, tag="pvT")
            nc.vector.tensor_copy(pvT[:], psT[:, :, 0:512])

            # MLP (feature-major): ph1 into banks 0..3
            for b in range(BL):
                nc.tensor.matmul(ps[:, b, :], w1pv[:], pvT[:, b, :],
                                 start=True, stop=False)
            for b in range(BL):
                nc.tensor.matmul(ps[:, b, :], w1ne[:], ne_t[:],
                                 start=False, stop=True)
            h1 = spool.tile([128, 4, 512], BF16, tag="h1")
            nc.scalar.activation(h1[:], ps[:, 0:4, :],
                                 mybir.ActivationFunctionType.Relu, bias=b1[:])
            for b in range(BL):
                nc.tensor.matmul(ps[:, b, :], w2t[:], h1[:, b, :],
                                 start=True, stop=True)
            t_sb = spool.tile([128, 4, 512], BF16, tag="t")
            nc.scalar.activation(t_sb[:], ps[:, 0:4, :],
                                 mybir.ActivationFunctionType.Identity,
                                 bias=b2[:])

            # transpose t back to token-major (bf16 psum, banks 4..5)
            t_tm = ps[:, 4:6, :].rearrange("p a e -> p (a e)").bitcast(
                BF16).rearrange("p (k b h) -> p k b h", k=4, b=BL)
            for b in range(BL):
                for kk in range(4):
                    nc.tensor.transpose(
                        t_tm[:, kk, b, :],
                        t_sb[:, b, kk * 128:(kk + 1) * 128], ident[:])

            # residual + store
            vt = spool.tile([128, 4, 512], BF16, tag="vt")
            nc.vector.tensor_add(
                vt[:].rearrange("p k (b h) -> p k b h", b=BL),
                pv_sb[:].rearrange("p k (b h) -> p k b h", b=BL), t_tm[:])
            dst = bufrows[bound:bound + N, :, :].rearrange(
                "(k p) b h -> p k (b h)", p=128)
            nc.sync.dma_start(dst, vt[:])

        def emit_steps():
            gtiles = {}
            emit_gather(0, 0, gtiles)
            for d in range(D):
                emit_gather(d, 1, gtiles)
                emit_compute(d, gtiles)
                if d + 1 < D:
                    emit_gather(d + 1, 0, gtiles)

        if repeat > 1:
            with tc.For_i(0, repeat, 1):
                emit_steps()
        else:
            emit_steps()

    nc.compile()
    return nc


# ----------------------------------------------------------------------------
# entry point
# ----------------------------------------------------------------------------

_CACHE = {}


def _get_compiled(parent_idx):
    key = hashlib.sha1(np.asarray(parent_idx).tobytes()).hexdigest()
    if key not in _CACHE:
        meta = build_layout(parent_idx)
        nc = build_nc(meta)
        _CACHE[key] = (nc, meta)
    return _CACHE[key]


def _prepare(inputs):
    embedding = np.asarray(inputs["embedding"], np.float32)
    emb_table = np.asarray(inputs["emb_table"], np.float32)
    W1 = np.asarray(inputs["W1"], np.float32)
    b1 = np.asarray(inputs["b1"], np.float32)
    W2 = np.asarray(inputs["W2"], np.float32)
    b2 = np.asarray(inputs["b2"], np.float32)
    parent_idx = np.asarray(inputs["parent_idx"])

    nc, meta = _get_compiled(parent_idx)
    sig = meta["sig"]

    embt = np.ascontiguousarray(
        np.concatenate([emb_table[2 + d * N + sig[d]] for d in range(D)],
                       axis=0).T).astype(bf16)          # [E, D*N]
    shared = dict(
        embt=embt,
        w1pv=np.ascontiguousarray(W1[:, :H].T).astype(bf16),
        w1ne=np.ascontiguousarray(W1[:, H:].T).astype(bf16),
        w2t=np.ascontiguousarray(W2.T).astype(bf16),
        b1=b1.reshape(H, 1).copy(),
        b2=b2.reshape(H, 1).copy(),
        ident=np.eye(H, dtype=np.float32).astype(bf16),
        idxs=meta["idx_packed"],
    )
    in_maps = []
    for c in range(NCORES):
        init2 = np.zeros((2, BL, H), bf16)
        init2[1] = embedding[c * BL:(c + 1) * BL].astype(bf16)
        in_maps.append(dict(shared, init2=init2))
    return nc, meta, in_maps


def _run(inputs, trace=False):
    nc, meta, in_maps = _prepare(inputs)
    res = run_bass_kernel_spmd(nc, in_maps, list(range(NCORES)), trace=trace)

    R = np.empty(TOTAL - 1, np.int64)
    R[0] = 1
    pos = meta["pos"]
    for d in range(D):
        R[1 + d * N: 1 + (d + 1) * N] = 2 + d * N + pos[d]
    out = np.empty((B, TOTAL - 1, H), np.float32)
    for c in range(NCORES):
        br = res.results[c]["bufrows"]                  # [TOTAL, BL, H] bf16
        out[c * BL:(c + 1) * BL] = br[R].transpose(1, 0, 2).astype(np.float32)
    return out, res


def kernel(**inputs) -> np.ndarray:
    out, _ = _run(inputs, trace=False)
    return out
